# revision 13
# baseline (speedup 1.0000x reference)
"""Trainium2 Bass kernel: multi-query prefix-LM attention.

Reference computation (B=2, N=2048, DIM=1024, HEADS=16, DH=64):
    q = einsum('bnd,hde->bhne', x, Wq) * DH**-0.5
    k, v = split(x @ Wkv)                       # single shared K/V head
    sim = q @ k^T,  masked where (j > i) & (j >= boundary[b])
    out = softmax(sim) @ v  -> concat heads -> @ Wo

Sharding: 8 cores = (batch b in {0,1}) x (query chunk qc in {0..3}, 512 rows
each).  Pure SPMD, zero collectives: every core redundantly projects K/V for
its batch (x^T is replicated per batch), computes 16-head attention for its
512 query rows, and writes a disjoint output slice.

Core dataflow (per core):
  - all matmuls run as out = lhsT.T @ rhs with fp32 data bitcast to float32r
    (full-rate on TRN2 when the moving free dim >= 256)
  - S^T tiles [128 kpos, 512 q] = kT_tile (stationary) x qT_head (moving);
    keys live on partitions so NO transpose of the softmax matrix is needed
  - exp runs on ScalarE straight out of PSUM in [128, 1536] batches
    (no max-subtraction: scores are ~N(0,1), |s| < ~7, exp is safe in fp32)
  - P.V uses v augmented with a ones-column as the stationary operand, so the
    softmax denominator falls out of the same matmul for free (row 64 of O^T)
  - the mask is additive bias data (host-built, -1e30) applied only to the
    key tiles that can contain masked columns given the actual boundaries
"""

import sys

import numpy as np

for _p in ("/opt/trn_rl_repo",):
    if _p not in sys.path:
        sys.path.insert(0, _p)

import concourse.bass as bass
import concourse.bacc as bacc_mod
import concourse.mybir as mybir
from concourse import bass_utils
from concourse.masks import make_identity
from concourse.tile import TileContext

B, N, DIM, HEADS, DH = 2, 2048, 1024, 16, 64
SCALE = DH ** -0.5
NCORES = 8
QCH = 512           # query rows per core
P = 128             # partitions
DT = DIM // P       # 8 dim tiles
NKT = N // P        # 16 key-position tiles
HPAIR = HEADS // 2  # 8 head pairs
NEG = np.float32(-1.0e30)

f32 = mybir.dt.float32
f32r = mybir.dt.float32r
bf16 = mybir.dt.bfloat16
EXP = mybir.ActivationFunctionType.Exp


def _build(bias_kts):
    """Build the (shared, SPMD) Bass program.

    bias_kts: sorted list of key-tile indices that receive the additive mask
    bias (derived on the host from the actual boundary values; always a
    suffix of range(16)).
    """
    nbk = len(bias_kts)
    nc = bacc_mod.Bacc("TRN2", target_bir_lowering=False, debug=False)

    xT = nc.dram_tensor("xT", [DIM, N], f32r, kind="ExternalInput")
    xq = nc.dram_tensor("xq", [DIM, QCH], f32r, kind="ExternalInput")
    wq = nc.dram_tensor("wq", [DIM, HEADS * DH], f32r, kind="ExternalInput")
    wkv = nc.dram_tensor("wkv", [DIM, 2 * DH], f32r, kind="ExternalInput")
    wo = nc.dram_tensor("wo", [HEADS * DH, DIM], f32r, kind="ExternalInput")
    biasT = nc.dram_tensor("biasT", [max(nbk, 1) * P, QCH], f32, kind="ExternalInput")

    def dep(eng, *aps):
        # single-wait dependency absorbers: walrus allows only ONE sync wait
        # per instruction, so fan many producer sems into nops, one each
        for ap in aps:
            with tc.tile_critical():
                nop = eng.nop(hint="dep").ins
                nop.ins = [eng.lower_ap(ap)]
    out = nc.dram_tensor("out", [QCH, DIM], f32, kind="ExternalOutput")

    with TileContext(nc) as tc:
        from contextlib import ExitStack

        with ExitStack() as ctx:
            big = ctx.enter_context(tc.tile_pool(name="big", bufs=1))
            wpool = ctx.enter_context(tc.tile_pool(name="wpool", bufs=1))
            ptp = ctx.enter_context(tc.tile_pool(name="ptp", bufs=2))
            small = ctx.enter_context(tc.tile_pool(name="small", bufs=2))
            obp = ctx.enter_context(tc.tile_pool(name="obp", bufs=8))
            stp = ctx.enter_context(tc.tile_pool(name="stp", bufs=2, space="PSUM"))
            otp = ctx.enter_context(tc.tile_pool(name="otp", bufs=2, space="PSUM"))

            # ---- loads ----------------------------------------------------
            xt_sb = big.tile([P, DT * N], f32r)
            for t in range(DT):
                nc.sync.dma_start(xt_sb[:, t * N:(t + 1) * N], xT[t * P:(t + 1) * P, :])
            xq_sb = wpool.tile([P, DT * QCH], f32r, tag="xo")
            for t in range(DT):
                nc.sync.dma_start(xq_sb[:, t * QCH:(t + 1) * QCH], xq[t * P:(t + 1) * P, :])
            wkv_sb = big.tile([P, DT * 2 * DH], f32r)
            for t in range(DT):
                nc.sync.dma_start(wkv_sb[:, t * 128:(t + 1) * 128], wkv[t * P:(t + 1) * P, :])
            bias_sb = big.tile([P, max(nbk, 1) * QCH], bf16)
            for k in range(nbk):
                bias_st = obp.tile([P, QCH], f32, tag="ob")
                nc.sync.dma_start(bias_st, biasT[k * P:(k + 1) * P, :])
                dep(nc.vector, bias_st[:, :])
                nc.vector.tensor_copy(bias_sb[:, k * QCH:(k + 1) * QCH], bias_st)
            ident = big.tile([P, P], f32)
            make_identity(nc, ident)

            # wq and wo share one slot (wq is dead before wo is needed)
            wq_sb = wpool.tile([P, DT * 1024], f32r, tag="w")
            for t in range(DT):
                nc.sync.dma_start(wq_sb[:, t * 1024:(t + 1) * 1024], wq[t * P:(t + 1) * P, :])

            # PE absorbs every input-load DMA queue sem, one nop per DMA
            dep(nc.tensor, *[xt_sb[:, t * N:(t + 1) * N] for t in range(DT)])
            dep(nc.tensor, *[xq_sb[:, t * QCH:(t + 1) * QCH] for t in range(DT)])
            dep(nc.tensor, *[wkv_sb[:, t * 128:(t + 1) * 128] for t in range(DT)])
            dep(nc.tensor, *[wq_sb[:, t * 1024:(t + 1) * 1024] for t in range(DT)])
            dep(nc.tensor, ident[:, :])

            # ---- projections ---------------------------------------------
            # qT: per head h -> [64 dh, 512 q], all heads at base partition 0
            qt_sb = big.tile([P, HEADS * QCH], bf16)
            for pr in range(HPAIR):
                qp = otp.tile([P, 512], f32, tag="ot")
                for t in range(DT):
                    nc.tensor.matmul(
                        qp,
                        lhsT=wq_sb[:, t * 1024 + pr * P: t * 1024 + (pr + 1) * P],
                        rhs=xq_sb[:, t * QCH:(t + 1) * QCH],
                        start=(t == 0), stop=(t == DT - 1),
                    )
                h0, h1 = 2 * pr, 2 * pr + 1
                nc.vector.tensor_copy(qt_sb[0:DH, h0 * QCH:(h0 + 1) * QCH], qp[0:DH, :])
                nc.vector.tensor_copy(qt_sb[DH:P, h1 * QCH:(h1 + 1) * QCH], qp[DH:P, :])
                nc.sync.dma_start(qt_sb[DH:P, h0 * QCH:(h0 + 1) * QCH],
                                  qt_sb[0:DH, h0 * QCH:(h0 + 1) * QCH])
                nc.sync.dma_start(qt_sb[0:DH, h1 * QCH:(h1 + 1) * QCH],
                                  qt_sb[DH:P, h1 * QCH:(h1 + 1) * QCH])

            # kvT: kT goes to bf16 [64, 2048]; vT half stays f32 for transposes
            kt_bf = big.tile([P, N], bf16)
            kvt_sb = big.tile([P, N], f32)
            for ch in range(N // 512):
                kp = otp.tile([P, 512], f32, tag="ot")
                for t in range(DT):
                    nc.tensor.matmul(
                        kp,
                        lhsT=wkv_sb[:, t * 128:(t + 1) * 128],
                        rhs=xt_sb[:, t * N + ch * 512: t * N + (ch + 1) * 512],
                        start=(t == 0), stop=(t == DT - 1),
                    )
                nc.vector.tensor_copy(kt_bf[0:DH, ch * 512:(ch + 1) * 512], kp[0:DH, :])
                nc.sync.dma_start(kt_bf[DH:P, ch * 512:(ch + 1) * 512],
                                  kt_bf[0:DH, ch * 512:(ch + 1) * 512])
                nc.vector.tensor_copy(kvt_sb[DH:128, ch * 512:(ch + 1) * 512], kp[DH:128, :])

            # v natural layout + ones column, bf16: [128 kpos, 65] per tile
            vaug_sb = big.tile([P, NKT * (DH + 1)], bf16)
            for kt in range(NKT):
                vp = otp.tile([P, DH], f32, tag="ot")
                nc.tensor.transpose(vp, kvt_sb[DH:128, kt * P:(kt + 1) * P], ident[DH:128, DH:128])
                nc.vector.tensor_copy(vaug_sb[:, kt * 65: kt * 65 + DH], vp)
                nc.vector.memset(vaug_sb[:, kt * 65 + DH: (kt + 1) * 65], 1.0)

            # ---- attention main loop -------------------------------------
            onT_sb = wpool.tile([P, HPAIR * QCH], f32r, tag="xo")
            groups = [list(range(s, min(s + 3, NKT))) for s in range(0, NKT, 3)]
            dep(nc.tensor, *[kt_bf[DH:P, ch * 512:(ch + 1) * 512]
                             for ch in range(N // 512)])
            dep(nc.tensor, *[qt_sb[:, h * QCH:(h + 1) * QCH]
                             for h in range(HEADS)])
            hist = {}
            for h in range(HEADS):
                if h >= 2:
                    sc2, bc2, on2 = hist[h - 2]
                    dep(nc.tensor, sc2)
                    dep(nc.scalar, sc2)
                    dep(nc.vector, bc2)
                    dep(nc.vector, on2)
                qTh = qt_sb  # sliced per row-group below
                pt = ptp.tile([P, NKT * QCH], bf16, tag="pt")
                ot = otp.tile([DH + 1, 512], f32, tag="ot")
                for kts in groups:
                    st = stp.tile([P, len(kts) * QCH], f32, tag="st")
                    for i, kt in enumerate(kts):
                        rg = (i % 2) * DH
                        nc.tensor.matmul(
                            st[:, i * QCH:(i + 1) * QCH],
                            lhsT=kt_bf[rg:rg + DH, kt * P:(kt + 1) * P],
                            rhs=qt_sb[rg:rg + DH, h * QCH:(h + 1) * QCH],
                            start=True, stop=True,
                        )
                    nc.scalar.activation(
                        pt[:, kts[0] * QCH: (kts[-1] + 1) * QCH],
                        st[:, 0: len(kts) * QCH],
                        EXP,
                    )
                    for kt in kts:
                        if kt in bias_kts:
                            bi = bias_kts.index(kt)
                            nc.vector.tensor_mul(
                                pt[:, kt * QCH:(kt + 1) * QCH],
                                pt[:, kt * QCH:(kt + 1) * QCH],
                                bias_sb[:, bi * QCH:(bi + 1) * QCH],
                            )
                for kt in range(NKT):
                    nc.tensor.matmul(
                        ot,
                        lhsT=vaug_sb[:, kt * 65:(kt + 1) * 65],
                        rhs=pt[:, kt * QCH:(kt + 1) * QCH],
                        start=(kt == 0), stop=(kt == NKT - 1),
                    )
                rec = small.tile([1, QCH], f32, tag="rec")
                nc.vector.reciprocal(rec, ot[DH:DH + 1, :])
                bc = small.tile([DH, QCH], f32, tag="bc")
                nc.gpsimd.partition_broadcast(bc, rec)
                sc = small.tile([DH, QCH], f32r, tag="sc")
                nc.vector.tensor_mul(sc, ot[0:DH, :], bc)
                on_slice = onT_sb[(h % 2) * DH:(h % 2) * DH + DH,
                                  (h // 2) * QCH:(h // 2 + 1) * QCH]
                nc.sync.dma_start(on_slice, sc)
                hist[h] = (sc[:, :], bc[:, :], on_slice)

            # ---- output projection ---------------------------------------
            wo_sb = wpool.tile([P, DT * 1024], f32r, tag="w")
            for t in range(DT):
                nc.sync.dma_start(wo_sb[:, t * 1024:(t + 1) * 1024], wo[t * P:(t + 1) * P, :])
            dep(nc.tensor, *[hist[h][2] for h in range(HEADS)])
            dep(nc.tensor, *[wo_sb[:, t * 1024:(t + 1) * 1024] for t in range(DT)])
            for tb in range(QCH // P):
                for chh in range(2):
                    op = stp.tile([P, 512], f32, tag="st")
                    for pr in range(HPAIR):
                        nc.tensor.matmul(
                            op,
                            lhsT=onT_sb[:, pr * QCH + tb * P: pr * QCH + (tb + 1) * P],
                            rhs=wo_sb[:, pr * 1024 + chh * 512: pr * 1024 + (chh + 1) * 512],
                            start=(pr == 0), stop=(pr == HPAIR - 1),
                        )
                    ob = obp.tile([P, 512], f32, tag="ob")
                    nc.vector.tensor_copy(ob, op)
                    nc.sync.dma_start(out[tb * P:(tb + 1) * P, chh * 512:(chh + 1) * 512], ob)

    return nc


def _prep_inputs(x, Wq, Wkv, Wo, bnd):
    """Host-side sharding: build the 8 per-core input dicts."""
    x = np.ascontiguousarray(np.asarray(x, dtype=np.float32))
    Wq = np.asarray(Wq, dtype=np.float32)
    Wkv = np.ascontiguousarray(np.asarray(Wkv, dtype=np.float32))
    Wo = np.ascontiguousarray(np.asarray(Wo, dtype=np.float32))
    bnd = np.asarray(bnd).astype(np.int64)

    wq_host = np.ascontiguousarray(
        (np.transpose(Wq, (1, 0, 2)).reshape(DIM, HEADS * DH) * SCALE).astype(np.float32))

    bias_kt_start = int(min(int(b) // P for b in bnd))
    bias_kts = list(range(bias_kt_start, NKT))
    nbk = len(bias_kts)

    in_maps = []
    for c in range(NCORES):
        b, qc = c // 4, c % 4
        xb = x[b]                                     # [N, DIM]
        xTb = np.ascontiguousarray(xb.T)              # [DIM, N]
        xqb = np.ascontiguousarray(xb[qc * QCH:(qc + 1) * QCH].T)  # [DIM, 512]
        iq = np.arange(qc * QCH, (qc + 1) * QCH)      # global query rows
        bias = np.ones((max(nbk, 1) * P, QCH), dtype=np.float32)
        for k, kt in enumerate(bias_kts):
            j = np.arange(kt * P, (kt + 1) * P)       # global key cols
            m = (j[:, None] > iq[None, :]) & (j[:, None] >= int(bnd[b]))
            bias[k * P:(k + 1) * P][m] = 0.0
        in_maps.append({
            "xT": xTb, "xq": xqb, "wq": wq_host, "wkv": Wkv, "wo": Wo,
            "biasT": bias,
        })
    return in_maps, bias_kts


_CACHE = {}


def kernel(x, Wq, Wkv, Wo, causal_boundary_indices, _trace=False):
    in_maps, bias_kts = _prep_inputs(x, Wq, Wkv, Wo, causal_boundary_indices)
    key = tuple(bias_kts)
    if key not in _CACHE:
        nc = _build(bias_kts)
        nc.finalize()
        _CACHE[key] = nc
    nc = _CACHE[key]
    res = bass_utils.run_bass_kernel_spmd(
        nc, in_maps, core_ids=list(range(NCORES)), trace=_trace,
    )
    out = np.empty((B, N, DIM), dtype=np.float32)
    for c in range(NCORES):
        b, qc = c // 4, c % 4
        out[b, qc * QCH:(qc + 1) * QCH, :] = res.results[c]["out"]
    if _trace:
        kernel._last = res
    return out


# revision 14
# speedup vs baseline: 1.0294x; 1.0294x over previous
"""Trainium2 Bass kernel: multi-query prefix-LM attention.

Reference computation (B=2, N=2048, DIM=1024, HEADS=16, DH=64):
    q = einsum('bnd,hde->bhne', x, Wq) * DH**-0.5
    k, v = split(x @ Wkv)                       # single shared K/V head
    sim = q @ k^T,  masked where (j > i) & (j >= boundary[b])
    out = softmax(sim) @ v  -> concat heads -> @ Wo

Sharding: 8 cores = (batch b in {0,1}) x (query chunk qc in {0..3}, 512 rows
each).  Pure SPMD, zero collectives: every core redundantly projects K/V for
its batch (x^T is replicated per batch), computes 16-head attention for its
512 query rows, and writes a disjoint output slice.

Core dataflow (per core):
  - projections and out-projection run in float32r end-to-end (full-rate on
    TRN2 when the moving free dim >= 256); QK and PV run bf16
  - S^T tiles [128 kpos, 512 q] = kT_tile (stationary) x qT_head (moving);
    keys live on partitions so NO transpose of the softmax matrix is needed.
    kT and qT are mirrored into both partition halves so consecutive score
    matmuls alternate PE row-groups 0/64 and run concurrently (K=64 packing);
    q is projected in head pairs (M=128) for the same reason
  - exp runs on ScalarE straight out of PSUM in [128, <=1536] batches
    (no max-subtraction: scores are ~N(0,1), |s| < ~7, exp is safe in fp32)
  - P.V uses v augmented with a ones-column as the stationary operand, so the
    softmax denominator falls out of the same matmul for free (row 64 of O^T)
  - the mask is multiplicative 0/1 bf16 data (host-built from the actual
    boundary values) applied to P after exp, only on key tiles that can
    contain masked columns; this keeps every matmul at <=1 semaphore wait
  - walrus here allows ONE sync wait per instruction: deps fan in through
    single-wait nop chains (dep()) and Bacc's wait-legalization passes
"""

import sys

import numpy as np

for _p in ("/opt/trn_rl_repo",):
    if _p not in sys.path:
        sys.path.insert(0, _p)

import concourse.bass as bass
import concourse.bacc as bacc_mod
import concourse.mybir as mybir
from concourse import bass_utils
from concourse.masks import make_identity
from concourse.tile import TileContext

B, N, DIM, HEADS, DH = 2, 2048, 1024, 16, 64
SCALE = DH ** -0.5
NCORES = 8
QCH = 512           # query rows per core
P = 128             # partitions
DT = DIM // P       # 8 dim tiles
NKT = N // P        # 16 key-position tiles
HPAIR = HEADS // 2  # 8 head pairs
NEG = np.float32(-1.0e30)

f32 = mybir.dt.float32
f32r = mybir.dt.float32r
bf16 = mybir.dt.bfloat16
EXP = mybir.ActivationFunctionType.Exp


def _build(bias_kts):
    """Build the (shared, SPMD) Bass program.

    bias_kts: sorted list of key-tile indices that receive the additive mask
    bias (derived on the host from the actual boundary values; always a
    suffix of range(16)).
    """
    nbk = len(bias_kts)
    nc = bacc_mod.Bacc("TRN2", target_bir_lowering=False, debug=False)

    xT = nc.dram_tensor("xT", [DIM, N], f32r, kind="ExternalInput")
    xq = nc.dram_tensor("xq", [DIM, QCH], f32r, kind="ExternalInput")
    wq = nc.dram_tensor("wq", [DIM, HEADS * DH], f32r, kind="ExternalInput")
    wkv = nc.dram_tensor("wkv", [DIM, 2 * DH], f32r, kind="ExternalInput")
    wo = nc.dram_tensor("wo", [HEADS * DH, DIM], f32r, kind="ExternalInput")
    biasT = nc.dram_tensor("biasT", [max(nbk, 1) * P, QCH], f32, kind="ExternalInput")

    def dep(eng, *aps):
        # single-wait dependency absorbers: walrus allows only ONE sync wait
        # per instruction, so fan many producer sems into nops, one each
        for ap in aps:
            with tc.tile_critical():
                nop = eng.nop(hint="dep").ins
                nop.ins = [eng.lower_ap(ap)]
    out = nc.dram_tensor("out", [QCH, DIM], f32, kind="ExternalOutput")

    with TileContext(nc) as tc:
        from contextlib import ExitStack

        with ExitStack() as ctx:
            big = ctx.enter_context(tc.tile_pool(name="big", bufs=1))
            wpool = ctx.enter_context(tc.tile_pool(name="wpool", bufs=1))
            ptp = ctx.enter_context(tc.tile_pool(name="ptp", bufs=2))
            small = ctx.enter_context(tc.tile_pool(name="small", bufs=2))
            obp = ctx.enter_context(tc.tile_pool(name="obp", bufs=8))
            stp = ctx.enter_context(tc.tile_pool(name="stp", bufs=2, space="PSUM"))
            otp = ctx.enter_context(tc.tile_pool(name="otp", bufs=2, space="PSUM"))

            # ---- loads ----------------------------------------------------
            xt_sb = big.tile([P, DT * N], f32r)
            for t in range(DT):
                nc.sync.dma_start(xt_sb[:, t * N:(t + 1) * N], xT[t * P:(t + 1) * P, :])
            xq_sb = wpool.tile([P, DT * QCH], f32r, tag="xo")
            for t in range(DT):
                nc.sync.dma_start(xq_sb[:, t * QCH:(t + 1) * QCH], xq[t * P:(t + 1) * P, :])
            wkv_sb = big.tile([P, DT * 2 * DH], f32r)
            for t in range(DT):
                nc.sync.dma_start(wkv_sb[:, t * 128:(t + 1) * 128], wkv[t * P:(t + 1) * P, :])
            bias_sb = big.tile([P, max(nbk, 1) * QCH], bf16)
            for k in range(nbk):
                bias_st = obp.tile([P, QCH], f32, tag="ob")
                nc.sync.dma_start(bias_st, biasT[k * P:(k + 1) * P, :])
                dep(nc.vector, bias_st[:, :])
                nc.vector.tensor_copy(bias_sb[:, k * QCH:(k + 1) * QCH], bias_st)
            ident = big.tile([P, P], f32)
            make_identity(nc, ident)

            # wq and wo share one slot (wq is dead before wo is needed)
            wq_sb = wpool.tile([P, DT * 1024], f32r, tag="w")
            for t in range(DT):
                nc.sync.dma_start(wq_sb[:, t * 1024:(t + 1) * 1024], wq[t * P:(t + 1) * P, :])

            # PE absorbs every input-load DMA queue sem, one nop per DMA
            dep(nc.tensor, *[xt_sb[:, t * N:(t + 1) * N] for t in range(DT)])
            dep(nc.tensor, *[xq_sb[:, t * QCH:(t + 1) * QCH] for t in range(DT)])
            dep(nc.tensor, *[wkv_sb[:, t * 128:(t + 1) * 128] for t in range(DT)])
            dep(nc.tensor, *[wq_sb[:, t * 1024:(t + 1) * 1024] for t in range(DT)])
            dep(nc.tensor, ident[:, :])

            # ---- projections ---------------------------------------------
            # qT: per head h -> [64 dh, 512 q], all heads at base partition 0
            qt_sb = big.tile([P, HEADS * QCH], bf16)
            for pr in range(HPAIR):
                qp = otp.tile([P, 512], f32, tag="ot")
                for t in range(DT):
                    nc.tensor.matmul(
                        qp,
                        lhsT=wq_sb[:, t * 1024 + pr * P: t * 1024 + (pr + 1) * P],
                        rhs=xq_sb[:, t * QCH:(t + 1) * QCH],
                        start=(t == 0), stop=(t == DT - 1),
                    )
                h0, h1 = 2 * pr, 2 * pr + 1
                nc.vector.tensor_copy(qt_sb[0:DH, h0 * QCH:(h0 + 1) * QCH], qp[0:DH, :])
                nc.vector.tensor_copy(qt_sb[DH:P, h1 * QCH:(h1 + 1) * QCH], qp[DH:P, :])
                nc.sync.dma_start(qt_sb[DH:P, h0 * QCH:(h0 + 1) * QCH],
                                  qt_sb[0:DH, h0 * QCH:(h0 + 1) * QCH])
                nc.sync.dma_start(qt_sb[0:DH, h1 * QCH:(h1 + 1) * QCH],
                                  qt_sb[DH:P, h1 * QCH:(h1 + 1) * QCH])

            # kvT: kT goes to bf16 [64, 2048]; vT half stays f32 for transposes
            kt_bf = big.tile([P, N], bf16)
            kvt_sb = big.tile([P, N], f32)
            for ch in range(N // 512):
                kp = otp.tile([P, 512], f32, tag="ot")
                for t in range(DT):
                    nc.tensor.matmul(
                        kp,
                        lhsT=wkv_sb[:, t * 128:(t + 1) * 128],
                        rhs=xt_sb[:, t * N + ch * 512: t * N + (ch + 1) * 512],
                        start=(t == 0), stop=(t == DT - 1),
                    )
                nc.vector.tensor_copy(kt_bf[0:DH, ch * 512:(ch + 1) * 512], kp[0:DH, :])
                nc.sync.dma_start(kt_bf[DH:P, ch * 512:(ch + 1) * 512],
                                  kt_bf[0:DH, ch * 512:(ch + 1) * 512])
                nc.vector.tensor_copy(kvt_sb[DH:128, ch * 512:(ch + 1) * 512], kp[DH:128, :])

            # v natural layout + ones column, bf16: [128 kpos, 65] per tile
            vaug_sb = big.tile([P, NKT * (DH + 1)], bf16)
            for kt in range(NKT):
                vp = otp.tile([P, DH], f32, tag="ot")
                nc.tensor.transpose(vp, kvt_sb[DH:128, kt * P:(kt + 1) * P], ident[DH:128, DH:128])
                nc.vector.tensor_copy(vaug_sb[:, kt * 65: kt * 65 + DH], vp)
                nc.vector.memset(vaug_sb[:, kt * 65 + DH: (kt + 1) * 65], 1.0)

            # ---- attention main loop -------------------------------------
            onT_sb = wpool.tile([P, HPAIR * QCH], f32r, tag="xo")
            groups = [list(range(s, min(s + 3, NKT))) for s in range(0, NKT, 3)]
            dep(nc.tensor, *[kt_bf[DH:P, ch * 512:(ch + 1) * 512]
                             for ch in range(N // 512)])
            dep(nc.tensor, *[qt_sb[:, h * QCH:(h + 1) * QCH]
                             for h in range(HEADS)])
            hist = {}
            for h in range(HEADS):
                if h >= 2:
                    sc2, bc2, on2 = hist[h - 2]
                    dep(nc.tensor, sc2)
                    dep(nc.scalar, sc2)
                    dep(nc.vector, bc2)
                    dep(nc.vector, on2)
                qTh = qt_sb  # sliced per row-group below
                pt = ptp.tile([P, NKT * QCH], bf16, tag="pt")
                ot = otp.tile([DH + 1, 512], f32, tag="ot")
                for kts in groups:
                    st = stp.tile([P, len(kts) * QCH], f32, tag="st")
                    for i, kt in enumerate(kts):
                        rg = (i % 2) * DH
                        nc.tensor.matmul(
                            st[:, i * QCH:(i + 1) * QCH],
                            lhsT=kt_bf[rg:rg + DH, kt * P:(kt + 1) * P],
                            rhs=qt_sb[rg:rg + DH, h * QCH:(h + 1) * QCH],
                            start=True, stop=True,
                        )
                    nc.scalar.activation(
                        pt[:, kts[0] * QCH: (kts[-1] + 1) * QCH],
                        st[:, 0: len(kts) * QCH],
                        EXP,
                    )
                    for kt in kts:
                        if kt in bias_kts:
                            bi = bias_kts.index(kt)
                            nc.vector.tensor_mul(
                                pt[:, kt * QCH:(kt + 1) * QCH],
                                pt[:, kt * QCH:(kt + 1) * QCH],
                                bias_sb[:, bi * QCH:(bi + 1) * QCH],
                            )
                for kt in range(NKT):
                    nc.tensor.matmul(
                        ot,
                        lhsT=vaug_sb[:, kt * 65:(kt + 1) * 65],
                        rhs=pt[:, kt * QCH:(kt + 1) * QCH],
                        start=(kt == 0), stop=(kt == NKT - 1),
                    )
                rec = small.tile([1, QCH], f32, tag="rec")
                nc.vector.reciprocal(rec, ot[DH:DH + 1, :])
                bc = small.tile([DH, QCH], f32, tag="bc")
                nc.gpsimd.partition_broadcast(bc, rec)
                sc = small.tile([DH, QCH], f32r, tag="sc")
                nc.vector.tensor_mul(sc, ot[0:DH, :], bc)
                on_slice = onT_sb[(h % 2) * DH:(h % 2) * DH + DH,
                                  (h // 2) * QCH:(h // 2 + 1) * QCH]
                nc.sync.dma_start(on_slice, sc)
                hist[h] = (sc[:, :], bc[:, :], on_slice)

            # ---- output projection ---------------------------------------
            wo_sb = wpool.tile([P, DT * 1024], f32r, tag="w")
            for t in range(DT):
                nc.sync.dma_start(wo_sb[:, t * 1024:(t + 1) * 1024], wo[t * P:(t + 1) * P, :])
            dep(nc.tensor, *[hist[h][2] for h in range(HEADS)])
            dep(nc.tensor, *[wo_sb[:, t * 1024:(t + 1) * 1024] for t in range(DT)])
            for tb in range(QCH // P):
                for chh in range(2):
                    op = stp.tile([P, 512], f32, tag="st")
                    for pr in range(HPAIR):
                        nc.tensor.matmul(
                            op,
                            lhsT=onT_sb[:, pr * QCH + tb * P: pr * QCH + (tb + 1) * P],
                            rhs=wo_sb[:, pr * 1024 + chh * 512: pr * 1024 + (chh + 1) * 512],
                            start=(pr == 0), stop=(pr == HPAIR - 1),
                        )
                    ob = obp.tile([P, 512], f32, tag="ob")
                    nc.vector.tensor_copy(ob, op)
                    nc.sync.dma_start(out[tb * P:(tb + 1) * P, chh * 512:(chh + 1) * 512], ob)

    return nc


def _prep_inputs(x, Wq, Wkv, Wo, bnd):
    """Host-side sharding: build the 8 per-core input dicts."""
    x = np.ascontiguousarray(np.asarray(x, dtype=np.float32))
    Wq = np.asarray(Wq, dtype=np.float32)
    Wkv = np.ascontiguousarray(np.asarray(Wkv, dtype=np.float32))
    Wo = np.ascontiguousarray(np.asarray(Wo, dtype=np.float32))
    bnd = np.asarray(bnd).astype(np.int64)

    wq_host = np.ascontiguousarray(
        (np.transpose(Wq, (1, 0, 2)).reshape(DIM, HEADS * DH) * SCALE).astype(np.float32))

    bias_kt_start = int(min(int(b) // P for b in bnd))
    bias_kts = list(range(bias_kt_start, NKT))
    nbk = len(bias_kts)

    in_maps = []
    for c in range(NCORES):
        b, qc = c // 4, c % 4
        xb = x[b]                                     # [N, DIM]
        xTb = np.ascontiguousarray(xb.T)              # [DIM, N]
        xqb = np.ascontiguousarray(xb[qc * QCH:(qc + 1) * QCH].T)  # [DIM, 512]
        iq = np.arange(qc * QCH, (qc + 1) * QCH)      # global query rows
        bias = np.ones((max(nbk, 1) * P, QCH), dtype=np.float32)
        for k, kt in enumerate(bias_kts):
            j = np.arange(kt * P, (kt + 1) * P)       # global key cols
            m = (j[:, None] > iq[None, :]) & (j[:, None] >= int(bnd[b]))
            bias[k * P:(k + 1) * P][m] = 0.0
        in_maps.append({
            "xT": xTb, "xq": xqb, "wq": wq_host, "wkv": Wkv, "wo": Wo,
            "biasT": bias,
        })
    return in_maps, bias_kts


_CACHE = {}


def kernel(x, Wq, Wkv, Wo, causal_boundary_indices, _trace=False):
    in_maps, bias_kts = _prep_inputs(x, Wq, Wkv, Wo, causal_boundary_indices)
    key = tuple(bias_kts)
    if key not in _CACHE:
        nc = _build(bias_kts)
        nc.finalize()
        _CACHE[key] = nc
    nc = _CACHE[key]
    res = bass_utils.run_bass_kernel_spmd(
        nc, in_maps, core_ids=list(range(NCORES)), trace=_trace,
    )
    out = np.empty((B, N, DIM), dtype=np.float32)
    for c in range(NCORES):
        b, qc = c // 4, c % 4
        out[b, qc * QCH:(qc + 1) * QCH, :] = res.results[c]["out"]
    if _trace:
        kernel._last = res
    return out
